# revision 1
# baseline (speedup 1.0000x reference)
"""Cumulative-min along time for trace[16, 8192, 256] on 8 TRN2 NeuronCores.

Strategy (data parallel, no collectives): shard the batch dim 16 -> 2 per
core. Per core, for each batch / 128-feature block, the [8192, 256] f32
slice is processed in groups of 512 time steps:

  DMA in  : natural layout tiles [128 time, 4, 256 feat]   (1KB rows)
  PE      : transpose 128x128 blocks  -> PSUM [128 feat, 512 time]
  DVE     : tensor_tensor_scan(min)   -> SBUF [128 feat, 512 time]
            (hardware prefix scan along free dim; carries chain groups)
  PE      : transpose back            -> PSUM [128 time, 4*256]
  ACT     : copy PSUM -> SBUF staging
  DMA out : natural layout

Everything is emitted under TileContext (auto semaphores / double
buffering). The kernel is compiled once per process and reused.
"""

import numpy as np

import concourse.bass as bass
import concourse.tile as tile
from concourse import bacc, masks, mybir
from concourse.bass_utils import run_bass_kernel_spmd

N_CORES = 8
B, T, F = 16, 8192, 256
B_LOC = B // N_CORES  # batches per core

P = 128          # partitions
GT = 512         # time steps per group (= one PSUM bank of f32)
CPG = GT // P    # chunks (of 128 time steps) per group
N_FB = F // P    # feature blocks
BIG = 3.0e38     # scan init: min(x, BIG) == x for all finite f32 inputs

F32 = mybir.dt.float32


def build_program(b_loc=B_LOC, t=T, f=F):
    n_groups = t // GT
    nc = bacc.Bacc("TRN2", target_bir_lowering=False, debug=False)
    x = nc.dram_tensor("trace", [b_loc, t, f], F32, kind="ExternalInput").ap()
    y = nc.dram_tensor("out", [b_loc, t, f], F32, kind="ExternalOutput").ap()

    with tile.TileContext(nc) as tc:
        with (
            tc.tile_pool(name="const", bufs=1) as const_pool,
            tc.tile_pool(name="ld", bufs=3) as ld_pool,
            tc.tile_pool(name="pa", bufs=4, space="PSUM") as pa_pool,
            tc.tile_pool(name="scn", bufs=3) as b_pool,
            tc.tile_pool(name="pc", bufs=2, space="PSUM") as pc_pool,
            tc.tile_pool(name="st", bufs=3) as d_pool,
        ):
            ident = const_pool.tile([P, P], F32)
            masks.make_identity(nc, ident[:])
            # data1 for the scan; ignored by op1=bypass but must have
            # matching dims and live in SBUF (data0 is in PSUM).
            dummy = const_pool.tile([P, GT], F32)
            nc.gpsimd.memset(dummy[:], 0.0)

            for b in range(b_loc):
                xb = x[b].rearrange("(g c p) f -> g p c f", p=P, c=CPG)
                yb = y[b].rearrange("(g c p) f -> g p c f", p=P, c=CPG)
                carries = [None] * N_FB
                for g in range(n_groups):
                    ld = ld_pool.tile([P, CPG, f], F32)
                    nc.sync.dma_start(out=ld[:], in_=xb[g])

                    bts = []
                    for fb in range(N_FB):
                        pa = pa_pool.tile([P, GT], F32)
                        for c in range(CPG):
                            nc.tensor.transpose(
                                out=pa[:, c * P:(c + 1) * P],
                                in_=ld[:, c, fb * P:(fb + 1) * P],
                                identity=ident[:],
                            )
                        bt = b_pool.tile([P, GT], F32)
                        init = carries[fb] if carries[fb] is not None else BIG
                        nc.vector.tensor_tensor_scan(
                            out=bt[:],
                            data0=pa[:],
                            data1=dummy[:],
                            initial=init,
                            op0=mybir.AluOpType.min,
                            op1=mybir.AluOpType.bypass,
                        )
                        carries[fb] = bt[:, GT - 1:GT]
                        bts.append(bt)

                    pc = pc_pool.tile([P, CPG, f], F32)
                    for fb in range(N_FB):
                        for c in range(CPG):
                            nc.tensor.transpose(
                                out=pc[:, c, fb * P:(fb + 1) * P],
                                in_=bts[fb][:, c * P:(c + 1) * P],
                                identity=ident[:],
                            )
                    d = d_pool.tile([P, CPG, f], F32)
                    nc.scalar.copy(out=d[:], in_=pc[:])
                    nc.sync.dma_start(out=yb[g], in_=d[:])

    nc.compile()
    return nc


_PROG = None


def _get_prog():
    global _PROG
    if _PROG is None:
        _PROG = build_program()
    return _PROG


def run(in_maps, **kwargs):
    nc = _get_prog()
    return run_bass_kernel_spmd(nc, in_maps, core_ids=list(range(N_CORES)), **kwargs)


def make_in_maps(trace):
    trace = np.ascontiguousarray(trace, dtype=np.float32)
    return [
        {"trace": trace[i * B_LOC:(i + 1) * B_LOC]} for i in range(N_CORES)
    ]


def kernel(trace):
    res = run(make_in_maps(trace))
    return np.concatenate([res.results[i]["out"] for i in range(N_CORES)], axis=0)


# revision 3
# speedup vs baseline: 1.3569x; 1.3569x over previous
"""Cumulative-min along time for trace[16, 8192, 256] on 8 TRN2 NeuronCores.

Data-parallel sharding (no collectives): batch dim 16 -> 2 per core.

The host prepares each core's shard in feature-major layout [2, 256, 8192]
(time contiguous), so on-device the cumulative min is a pure streaming
workload: DMA a [128 lanes, TT time] tile in, run the DVE hardware prefix
scan (tensor_tensor_scan with op=min) along the free dim, DMA out.
Carries chain consecutive time chunks per lane. No tensor-engine work,
no PSUM: the kernel runs at the HBM roofline. The host transposes the
result back to [b, t, f] while gathering.
"""

import numpy as np

import concourse.bass as bass
import concourse.tile as tile
from concourse import bacc, mybir
from concourse.bass_utils import run_bass_kernel_spmd

N_CORES = 8
B, T, F = 16, 8192, 256
B_LOC = B // N_CORES  # batches per core

P = 128          # partitions (lanes per tile)
TT = 2048        # time steps per scan chunk
BIG = 3.0e38     # scan init: min(x, BIG) == x for all finite f32 inputs

F32 = mybir.dt.float32


def build_program(b_loc=B_LOC, t=T, f=F, tt=TT):
    lanes = b_loc * f
    n_lt = lanes // P        # lane tiles
    n_c = t // tt            # time chunks per lane
    nc = bacc.Bacc("TRN2", target_bir_lowering=False, debug=False)
    x = nc.dram_tensor("trace", [lanes, t], F32, kind="ExternalInput").ap()
    y = nc.dram_tensor("out", [lanes, t], F32, kind="ExternalOutput").ap()

    with tile.TileContext(nc) as tc:
        with (
            tc.tile_pool(name="const", bufs=1) as const_pool,
            tc.tile_pool(name="ld", bufs=4) as ld_pool,
            # res tiles double as carry sources for the next chunk of the
            # same lane tile, so one slot per concurrently-live chain plus
            # slack for store overlap.
            tc.tile_pool(name="res", bufs=8) as res_pool,
        ):
            # data1 for the scan; ignored by op1=bypass but must have
            # matching dims.
            dummy = const_pool.tile([P, tt], F32)
            nc.gpsimd.memset(dummy[:], 0.0)

            carries = [None] * n_lt
            for c in range(n_c):
                for lt in range(n_lt):
                    ld = ld_pool.tile([P, tt], F32)
                    nc.sync.dma_start(
                        out=ld[:],
                        in_=x[lt * P:(lt + 1) * P, c * tt:(c + 1) * tt],
                    )
                    res = res_pool.tile([P, tt], F32)
                    init = carries[lt] if carries[lt] is not None else BIG
                    nc.vector.tensor_tensor_scan(
                        out=res[:],
                        data0=ld[:],
                        data1=dummy[:],
                        initial=init,
                        op0=mybir.AluOpType.min,
                        op1=mybir.AluOpType.bypass,
                    )
                    carries[lt] = res[:, tt - 1:tt]
                    nc.sync.dma_start(
                        out=y[lt * P:(lt + 1) * P, c * tt:(c + 1) * tt],
                        in_=res[:],
                    )

    nc.compile()
    return nc


_PROG = None


def _get_prog():
    global _PROG
    if _PROG is None:
        _PROG = build_program()
    return _PROG


def run(in_maps, **kwargs):
    nc = _get_prog()
    return run_bass_kernel_spmd(nc, in_maps, core_ids=list(range(N_CORES)), **kwargs)


def make_in_maps(trace):
    trace = np.asarray(trace, dtype=np.float32)
    maps = []
    for i in range(N_CORES):
        shard = trace[i * B_LOC:(i + 1) * B_LOC]          # [2, T, F]
        shard = np.ascontiguousarray(shard.transpose(0, 2, 1))  # [2, F, T]
        maps.append({"trace": shard.reshape(B_LOC * F, T)})
    return maps


def kernel(trace):
    res = run(make_in_maps(trace))
    parts = []
    for i in range(N_CORES):
        o = res.results[i]["out"].reshape(B_LOC, F, T)
        parts.append(o.transpose(0, 2, 1))                # [2, T, F]
    return np.ascontiguousarray(np.concatenate(parts, axis=0))
